# revision 17
# baseline (speedup 1.0000x reference)
"""Trainium2 Bass kernel for CSAM channel self-attention module.

Per batch b (one per NeuronCore, B=8 over 8 cores):
    v      = x2[b].reshape(7, D)                 # D = 64*128*128 = 1048576
    E      = v @ v.T                             # [7,7] gram ("energy")
    att    = softmax(rowmax(E) - E, axis=-1)     # == exp(rowmin(E)-E)/Z
    out    = att @ v
    y[b]   = x1[b] * (gamma*out) + x1[b] = x1[b] * (gamma*out + 1)

Layout: d = q*65536 + w*2048 + f  (Q=16 runs, stream tiles [112, 2048] with
partition p = 7*q + m and 8KB contiguous DRAM lines).

Pass A: stream x2 full tiles [112, 2048] rotating over THREE DMA queues
(sync/scalar HWDGE + gpsimd SWDGE) — two queues cap at ~143 GB/s each, three
reach the ~345 GB/s SDMA-engine limit.  DVE casts fp32 -> fp16 cache tiles
(row 112 = 1.0 for the fused "+1"), PE transposes [112,128] chunks -> PSUM
[128,112] fp16, ACT copies them to SBUF (keeping DVE free for casts),
gram-matmuls accumulate E_psum[112,112] (diag 7x7 blocks = per-q partial
gram), trailing the transposes by a few chunks so the in-order PE queue
never stalls on the copy round-trip.  fp16 is safe: top-2 energy gaps are
>100 while fp16 gram error is ~+-2.

Energy -> weights entirely on-chip (no DRAM bounces): 16 identity-slice
matmuls extract+sum the diag blocks (e7[n,m] = sum_q E[7q+n,7q+m]), softmax
on [7,7], then W_psum = L7^T @ (bcast gamma*att^T) replicates the block to
all 16 q-positions and a block-diag mask multiply zeroes the off-diag
blocks while copying to SBUF.  wt row 112 = 1.0 preset at init.

Pass B: out_psum[112,512] = W.T @ Xh slices (fp16, weights resident), then
y = out_psum * x1 on DVE, staged into [112,2048] tiles; x1 loads and y
stores each rotate over all three DMA queues.  x2 is read from HBM exactly
once; x1 prefetch (bufs=3) covers the short softmax gap.
"""

import sys

import numpy as np

try:
    import concourse.bass as bass
except ImportError:  # grading env fallback
    sys.path.insert(0, "/opt/trn_rl_repo")
    import concourse.bass as bass

from contextlib import ExitStack

import concourse.bacc as bacc
import concourse.tile as tile
from concourse import mybir
from concourse.bass_utils import run_bass_kernel_spmd
from concourse.masks import make_identity

F32 = mybir.dt.float32
F16 = mybir.dt.float16

B = 8
NN = 7              # attention dim
Q = 16              # d-runs per channel
P = NN * Q          # 112 partitions of (q, m)
PK = P + 1          # +1 ones row for the fused "+1"
FS = 2048           # stream tile free dim (8KB DRAM lines)
FM = 512            # matmul slice free dim (one PSUM bank)
D_FULL = 64 * 128 * 128
N_CORES = 8
PIPE = 16           # gram matmul trails transposes by this many chunks
GRP = 8             # transpose chunks batched per PSUM tile
NXT = 4             # x2 stream slots (half-tiles, decoupled from queues)


def build_nc(d_total=D_FULL):
    assert d_total % (Q * FS) == 0
    ws = d_total // (Q * FS)          # stream tiles (32 at full size)
    cpt = FS // 128                   # transpose chunks per stream tile (16)
    mpt = FS // FM                    # matmul slices per stream tile (4)
    n_gram = ws * cpt

    nc = bacc.Bacc("TRN2", target_bir_lowering=False, debug=False)
    x1 = nc.dram_tensor("x1", [NN, d_total], F32, kind="ExternalInput")
    x2 = nc.dram_tensor("x2", [NN, d_total], F32, kind="ExternalInput")
    gm = nc.dram_tensor("gamma", [1], F32, kind="ExternalInput")
    y = nc.dram_tensor("y", [NN, d_total], F32, kind="ExternalOutput")

    x2v = x2[:].rearrange("m (q w f) -> q m w f", q=Q, w=ws, f=FS)
    x1v = x1[:].rearrange("m (q w f) -> q m w f", q=Q, w=ws, f=FS)
    yv = y[:].rearrange("m (q w f) -> q m w f", q=Q, w=ws, f=FS)

    with tile.TileContext(nc) as tc, ExitStack() as ctx:
        consts = ctx.enter_context(tc.tile_pool(name="consts", bufs=1))
        cache = ctx.enter_context(tc.tile_pool(name="cache", bufs=1))
        xs = ctx.enter_context(tc.tile_pool(name="xs", bufs=1))
        x1s = ctx.enter_context(tc.tile_pool(name="x1s", bufs=3))
        tsb = ctx.enter_context(tc.tile_pool(name="tsb", bufs=3))
        ys = ctx.enter_context(tc.tile_pool(name="ys", bufs=3))
        small = ctx.enter_context(tc.tile_pool(name="small", bufs=1))
        tps = ctx.enter_context(tc.tile_pool(name="tps", bufs=3, space="PSUM"))
        eps = ctx.enter_context(tc.tile_pool(name="eps", bufs=1, space="PSUM"))
        ops = ctx.enter_context(tc.tile_pool(name="ops", bufs=3, space="PSUM"))

        ident = consts.tile([P, P], F16)
        make_identity(nc, ident)
        ident32 = consts.tile([P, P], F32)
        make_identity(nc, ident32)
        ones32 = consts.tile([1, FS // 2], F32)
        nc.vector.memset(ones32[:], 1.0)

        # block-diag ones mask [112,112]: mask[7q+m, 7q'+n] = (q == q').
        # Built as S^T S with S[k, 7q+m] = (k == q) (q-selector, from ident32).
        sel = consts.tile([Q, P], F32)
        nc.vector.tensor_copy(
            out=sel[:],
            in_=bass.AP(
                tensor=ident32.tensor, offset=ident32.offset,
                ap=[[ident32.ap[0][0], Q], [1, Q], [0, NN]],
            ),
        )
        mask_ps = ops.tile([P, P], F32, tag="op")
        nc.tensor.matmul(mask_ps[:], lhsT=sel[:], rhs=sel[:],
                         start=True, stop=True)
        mask = consts.tile([P, P], F32)
        nc.vector.tensor_copy(out=mask[:], in_=mask_ps[:])

        # L7 [7, 112] fp16: L7[k, 7q+m] = (k == m)  (tiled identity)
        l7 = consts.tile([NN, P], F16)
        nc.vector.tensor_copy(
            out=l7[:],
            in_=bass.AP(
                tensor=ident.tensor, offset=ident.offset,
                ap=[[ident.ap[0][0], NN], [0, Q], [1, NN]],
            ),
        )

        # weights tile for pass B; ones row preset (DMA — compute engines
        # cannot address a base partition of 112), body written after softmax
        wt = small.tile([PK, P], F16)
        nc.gpsimd.dma_start(out=wt[P:PK, :], in_=ones32[0:1, 0:P])

        # gamma broadcast to 7 partitions
        gsb = small.tile([NN, 1], F32)
        nc.gpsimd.dma_start(
            out=gsb[:],
            in_=bass.AP(tensor=gm[:].tensor, offset=0, ap=[[0, NN], [1, 1]]),
        )

        E = eps.tile([P, P], F32)
        xh = [cache.tile([PK, FS], F16, name=f"xh{w}", tag=f"xh{w}")
              for w in range(ws)]

        # ~3us of dummy matmuls so the PE HAM clock-gate opens before the
        # real pass-A stream arrives (and stays open)
        for _ in range(30):
            wm = ops.tile([P, P], F32, tag="op")
            nc.tensor.matmul(wm[:], lhsT=ident[:], rhs=ident[:],
                             start=True, stop=True)

        # ---------------- pass A: stream x2, cast, transpose, gram ----------
        pend = []          # tt slices awaiting gram matmul
        gi = 0             # gram matmuls emitted

        def emit_gram(tt_ap):
            nonlocal gi
            nc.tensor.matmul(E[:], lhsT=tt_ap, rhs=tt_ap,
                             start=(gi == 0), stop=(gi == n_gram - 1))
            gi += 1

        # persistent stream slots: row 112 holds 1.0, copied along by the cast
        xts = [xs.tile([PK, FS // 2], F32, name=f"xt{i}", tag=f"xt{i}", bufs=1)
               for i in range(NXT)]
        for i in range(NXT):
            nc.gpsimd.dma_start(out=xts[i][P:PK, :], in_=ones32[0:1, :])

        HF = FS // 2
        for w in range(ws):
            for h in range(2):
                i = 2 * w + h
                hs = slice(h * HF, (h + 1) * HF)
                xt = xts[i % NXT]
                dmae = (nc.sync, nc.scalar, nc.gpsimd)[i % 3]
                dmae.dma_start(out=xt[0:P, :], in_=x2v[:, :, w, hs])
                nc.vector.tensor_copy(out=xh[w][:, hs], in_=xt[:])  # +cast
                tp = tps.tile([128, GRP * P], F16)
                for k in range(GRP):
                    c = h * GRP + k
                    nc.tensor.transpose(
                        tp[:, k * P:(k + 1) * P],
                        xh[w][0:P, c * 128:(c + 1) * 128], ident[:])
                tt = tsb.tile([128, GRP * P], F16)
                nc.vector.tensor_copy(out=tt[:], in_=tp[:])
                for k in range(GRP):
                    pend.append(tt[:, k * P:(k + 1) * P])
                while len(pend) > PIPE:
                    emit_gram(pend.pop(0))
        for tt in pend:
            emit_gram(tt)
        pend = []

        # ---------------- energy -> attention -> weights (all on-chip) ------
        e_sb = small.tile([P, P], F32)
        nc.scalar.copy(e_sb[:], E[:])                  # PSUM -> SBUF
        # e7[n,m] = sum_q E[7q+n, 7q+m]: 16 identity-slice matmuls.
        # Reuses the (now consumed) gram bank — start=True clears it.
        e7 = E[0:NN, 0:NN]
        for q in range(Q):
            s = slice(NN * q, NN * (q + 1))
            nc.tensor.matmul(e7[:], lhsT=ident32[:, s], rhs=e_sb[:, s],
                             start=(q == 0), stop=(q == Q - 1))
        mn = small.tile([NN, 1], F32)
        nc.vector.tensor_reduce(
            out=mn[:], in_=e7[:], axis=mybir.AxisListType.X,
            op=mybir.AluOpType.min,
        )
        ex = small.tile([NN, NN], F32)
        nc.scalar.activation(
            out=ex[:], in_=e7[:], func=mybir.ActivationFunctionType.Exp,
            bias=mn[:], scale=-1.0,
        )                                              # exp(rowmin - E)
        z = small.tile([NN, 1], F32)
        nc.vector.tensor_reduce(
            out=z[:], in_=ex[:], axis=mybir.AxisListType.X,
            op=mybir.AluOpType.add,
        )
        r = small.tile([NN, 1], F32)
        nc.vector.reciprocal(r[:], z[:])
        rg = small.tile([NN, 1], F32)
        nc.vector.tensor_mul(rg[:], r[:], gsb[:])      # gamma / Z_n
        a32 = small.tile([32, 32], F32)
        nc.vector.memset(a32[:], 0.0)
        nc.vector.tensor_scalar_mul(a32[0:NN, 0:NN], ex[:], rg[:])  # gamma*att
        at32 = small.tile([32, 32], F32)
        nc.vector.transpose(at32[:], a32[:])           # (gamma*att)^T
        # at16[k, 7q'+n] = at32[k, n]: broadcast along q', cast to fp16
        at16 = small.tile([NN, P], F16)
        nc.vector.tensor_copy(
            out=at16[:],
            in_=bass.AP(
                tensor=at32.tensor, offset=at32.offset,
                ap=[[at32.ap[0][0], NN], [0, Q], [1, NN]],
            ),
        )
        # W_ps[7q+m, 7q'+n] = at32[m, n] for every (q, q') block.
        # Reuses the gram bank again (e7 fully consumed by the exp above).
        w_ps = E[:]
        nc.tensor.matmul(w_ps, lhsT=l7[:], rhs=at16[:],
                         start=True, stop=True)
        # block-diag mask zeroes q != q' while copying PSUM -> SBUF fp16
        nc.vector.tensor_mul(wt[0:P, :], w_ps, mask[:])

        # ---------------- pass B: out = W.T @ Xh; y = out * x1 --------------
        for w in range(ws):
            x1t = x1s.tile([P, FS], F32)
            x1e = nc.scalar if w % 2 == 0 else nc.sync
            x1e.dma_start(out=x1t[:], in_=x1v[:, :, w, :])
            yt = ys.tile([P, FS], F32)
            for j in range(mpt):
                sl = slice(j * FM, (j + 1) * FM)
                op = ops.tile([P, FM], F32, tag="op")
                nc.tensor.matmul(op[:], lhsT=wt[:], rhs=xh[w][:, sl],
                                 start=True, stop=True)
                nc.vector.tensor_mul(yt[:, sl], op[:], x1t[:, sl])
            ye = (nc.gpsimd, nc.sync, nc.scalar)[w % 3]
            ye.dma_start(out=yv[:, :, w, :], in_=yt[:])

    nc.compile()
    return nc


_NC_CACHE = {}


def _get_nc(d_total=D_FULL):
    if d_total not in _NC_CACHE:
        _NC_CACHE[d_total] = build_nc(d_total)
    return _NC_CACHE[d_total]


def kernel(x1: np.ndarray, x2: np.ndarray, gamma: np.ndarray) -> np.ndarray:
    b, n, c, h, w = x1.shape
    assert (b, n) == (B, NN)
    d = c * h * w
    x1r = np.ascontiguousarray(x1.reshape(b, n, d)).astype(np.float32, copy=False)
    x2r = np.ascontiguousarray(x2.reshape(b, n, d)).astype(np.float32, copy=False)
    g = np.asarray(gamma, dtype=np.float32).reshape(1)

    nc = _get_nc(d)
    in_maps = [
        {"x1": x1r[i], "x2": x2r[i], "gamma": g} for i in range(N_CORES)
    ]
    res = run_bass_kernel_spmd(nc, in_maps, list(range(N_CORES)))
    out = np.stack([res.results[i]["y"] for i in range(N_CORES)], axis=0)
    return out.reshape(b, n, c, h, w).astype(np.float32, copy=False)


# revision 18
# speedup vs baseline: 1.0703x; 1.0703x over previous
"""Trainium2 Bass kernel for CSAM channel self-attention module.

Per batch b (one per NeuronCore, B=8 over 8 cores):
    v      = x2[b].reshape(7, D)                 # D = 64*128*128 = 1048576
    E      = v @ v.T                             # [7,7] gram ("energy")
    att    = softmax(rowmax(E) - E, axis=-1)     # == exp(rowmin(E)-E)/Z
    out    = att @ v
    y[b]   = x1[b] * (gamma*out) + x1[b] = x1[b] * (gamma*out + 1)

Layout: d = q*65536 + w*2048 + f  (Q=16 runs, stream tiles [112, 2048] with
partition p = 7*q + m and 8KB contiguous DRAM lines).

Pass A: stream x2 full tiles [112, 2048] rotating over THREE DMA queues
(sync/scalar HWDGE + gpsimd SWDGE) — two queues cap at ~143 GB/s each, three
reach the ~345 GB/s SDMA-engine limit.  DVE casts fp32 -> fp16 cache tiles
(row 112 = 1.0 for the fused "+1"), PE transposes [112,128] chunks -> PSUM
[128,112] fp16, ACT copies them to SBUF (keeping DVE free for casts),
gram-matmuls accumulate E_psum[112,112] (diag 7x7 blocks = per-q partial
gram), trailing the transposes by a few chunks so the in-order PE queue
never stalls on the copy round-trip.  fp16 is safe: top-2 energy gaps are
>100 while fp16 gram error is ~+-2.

Energy -> weights entirely on-chip (no DRAM bounces): 16 identity-slice
matmuls extract+sum the diag blocks (e7[n,m] = sum_q E[7q+n,7q+m]), softmax
on [7,7], then W_psum = L7^T @ (bcast gamma*att^T) replicates the block to
all 16 q-positions and a block-diag mask multiply zeroes the off-diag
blocks while copying to SBUF.  wt row 112 = 1.0 preset at init.

Pass B: out_psum[112,512] = W.T @ Xh slices (fp16, weights resident), then
y = out_psum * x1 on DVE, staged into [112,2048] tiles; x1 loads and y
stores each rotate over all three DMA queues.  x2 is read from HBM exactly
once; x1 prefetch (bufs=3) covers the short softmax gap.
"""

import sys

import numpy as np

try:
    import concourse.bass as bass
except ImportError:  # grading env fallback
    sys.path.insert(0, "/opt/trn_rl_repo")
    import concourse.bass as bass

from contextlib import ExitStack

import concourse.bacc as bacc
import concourse.tile as tile
from concourse import mybir
from concourse.bass_utils import run_bass_kernel_spmd
from concourse.masks import make_identity

F32 = mybir.dt.float32
F16 = mybir.dt.float16

B = 8
NN = 7              # attention dim
Q = 16              # d-runs per channel
P = NN * Q          # 112 partitions of (q, m)
PK = P + 1          # +1 ones row for the fused "+1"
FS = 2048           # stream tile free dim (8KB DRAM lines)
FM = 512            # matmul slice free dim (one PSUM bank)
D_FULL = 64 * 128 * 128
N_CORES = 8
PIPE = 8            # gram matmul trails transposes by this many chunks
GRP = 8             # transpose chunks batched per PSUM tile
NXT = 5             # x2 stream slots (half-tiles, decoupled from queues)


def build_nc(d_total=D_FULL):
    assert d_total % (Q * FS) == 0
    ws = d_total // (Q * FS)          # stream tiles (32 at full size)
    cpt = FS // 128                   # transpose chunks per stream tile (16)
    mpt = FS // FM                    # matmul slices per stream tile (4)
    n_gram = ws * cpt

    nc = bacc.Bacc("TRN2", target_bir_lowering=False, debug=False)
    x1 = nc.dram_tensor("x1", [NN, d_total], F32, kind="ExternalInput")
    x2 = nc.dram_tensor("x2", [NN, d_total], F32, kind="ExternalInput")
    gm = nc.dram_tensor("gamma", [1], F32, kind="ExternalInput")
    y = nc.dram_tensor("y", [NN, d_total], F32, kind="ExternalOutput")

    x2v = x2[:].rearrange("m (q w f) -> q m w f", q=Q, w=ws, f=FS)
    x1v = x1[:].rearrange("m (q w f) -> q m w f", q=Q, w=ws, f=FS)
    yv = y[:].rearrange("m (q w f) -> q m w f", q=Q, w=ws, f=FS)

    with tile.TileContext(nc) as tc, ExitStack() as ctx:
        consts = ctx.enter_context(tc.tile_pool(name="consts", bufs=1))
        cache = ctx.enter_context(tc.tile_pool(name="cache", bufs=1))
        xs = ctx.enter_context(tc.tile_pool(name="xs", bufs=1))
        x1s = ctx.enter_context(tc.tile_pool(name="x1s", bufs=3))
        tsb = ctx.enter_context(tc.tile_pool(name="tsb", bufs=2))
        ys = ctx.enter_context(tc.tile_pool(name="ys", bufs=3))
        small = ctx.enter_context(tc.tile_pool(name="small", bufs=1))
        tps = ctx.enter_context(tc.tile_pool(name="tps", bufs=3, space="PSUM"))
        eps = ctx.enter_context(tc.tile_pool(name="eps", bufs=1, space="PSUM"))
        ops = ctx.enter_context(tc.tile_pool(name="ops", bufs=3, space="PSUM"))

        ident = consts.tile([P, P], F16)
        make_identity(nc, ident)
        ident32 = consts.tile([P, P], F32)
        make_identity(nc, ident32)
        ones32 = consts.tile([1, FS // 2], F32)
        nc.vector.memset(ones32[:], 1.0)

        # block-diag ones mask [112,112]: mask[7q+m, 7q'+n] = (q == q').
        # Built as S^T S with S[k, 7q+m] = (k == q) (q-selector, from ident32).
        sel = consts.tile([Q, P], F32)
        nc.vector.tensor_copy(
            out=sel[:],
            in_=bass.AP(
                tensor=ident32.tensor, offset=ident32.offset,
                ap=[[ident32.ap[0][0], Q], [1, Q], [0, NN]],
            ),
        )
        mask_ps = ops.tile([P, P], F32, tag="op")
        nc.tensor.matmul(mask_ps[:], lhsT=sel[:], rhs=sel[:],
                         start=True, stop=True)
        mask = consts.tile([P, P], F32)
        nc.vector.tensor_copy(out=mask[:], in_=mask_ps[:])

        # L7 [7, 112] fp16: L7[k, 7q+m] = (k == m)  (tiled identity)
        l7 = consts.tile([NN, P], F16)
        nc.vector.tensor_copy(
            out=l7[:],
            in_=bass.AP(
                tensor=ident.tensor, offset=ident.offset,
                ap=[[ident.ap[0][0], NN], [0, Q], [1, NN]],
            ),
        )

        # weights tile for pass B; ones row preset (DMA — compute engines
        # cannot address a base partition of 112), body written after softmax
        wt = small.tile([PK, P], F16)
        nc.gpsimd.dma_start(out=wt[P:PK, :], in_=ones32[0:1, 0:P])

        # gamma broadcast to 7 partitions
        gsb = small.tile([NN, 1], F32)
        nc.gpsimd.dma_start(
            out=gsb[:],
            in_=bass.AP(tensor=gm[:].tensor, offset=0, ap=[[0, NN], [1, 1]]),
        )

        E = eps.tile([P, P], F32)
        xh = [cache.tile([PK, FS], F16, name=f"xh{w}", tag=f"xh{w}")
              for w in range(ws)]

        # ~3us of dummy matmuls so the PE HAM clock-gate opens before the
        # real pass-A stream arrives (and stays open)
        for _ in range(30):
            wm = ops.tile([P, P], F32, tag="op")
            nc.tensor.matmul(wm[:], lhsT=ident[:], rhs=ident[:],
                             start=True, stop=True)

        # ---------------- pass A: stream x2, cast, transpose, gram ----------
        pend = []          # tt slices awaiting gram matmul
        gi = 0             # gram matmuls emitted

        def emit_gram(tt_ap):
            nonlocal gi
            nc.tensor.matmul(E[:], lhsT=tt_ap, rhs=tt_ap,
                             start=(gi == 0), stop=(gi == n_gram - 1))
            gi += 1

        # persistent stream slots: row 112 holds 1.0, copied along by the cast
        xts = [xs.tile([PK, FS // 2], F32, name=f"xt{i}", tag=f"xt{i}", bufs=1)
               for i in range(NXT)]
        for i in range(NXT):
            nc.gpsimd.dma_start(out=xts[i][P:PK, :], in_=ones32[0:1, :])

        HF = FS // 2
        for w in range(ws):
            for h in range(2):
                i = 2 * w + h
                hs = slice(h * HF, (h + 1) * HF)
                xt = xts[i % NXT]
                dmae = (nc.sync, nc.scalar, nc.gpsimd)[i % 3]
                dmae.dma_start(out=xt[0:P, :], in_=x2v[:, :, w, hs])
                nc.vector.tensor_copy(out=xh[w][:, hs], in_=xt[:])  # +cast
                tp = tps.tile([128, GRP * P], F16)
                for k in range(GRP):
                    c = h * GRP + k
                    nc.tensor.transpose(
                        tp[:, k * P:(k + 1) * P],
                        xh[w][0:P, c * 128:(c + 1) * 128], ident[:])
                tt = tsb.tile([128, GRP * P], F16)
                nc.vector.tensor_copy(out=tt[:], in_=tp[:])
                for k in range(GRP):
                    pend.append(tt[:, k * P:(k + 1) * P])
                while len(pend) > PIPE:
                    emit_gram(pend.pop(0))
        for tt in pend:
            emit_gram(tt)
        pend = []

        # ---------------- energy -> attention -> weights (all on-chip) ------
        e_sb = small.tile([P, P], F32)
        nc.scalar.copy(e_sb[:], E[:])                  # PSUM -> SBUF
        # e7[n,m] = sum_q E[7q+n, 7q+m]: 16 identity-slice matmuls.
        # Reuses the (now consumed) gram bank — start=True clears it.
        e7 = E[0:NN, 0:NN]
        for q in range(Q):
            s = slice(NN * q, NN * (q + 1))
            nc.tensor.matmul(e7[:], lhsT=ident32[:, s], rhs=e_sb[:, s],
                             start=(q == 0), stop=(q == Q - 1))
        mn = small.tile([NN, 1], F32)
        nc.vector.tensor_reduce(
            out=mn[:], in_=e7[:], axis=mybir.AxisListType.X,
            op=mybir.AluOpType.min,
        )
        ex = small.tile([NN, NN], F32)
        nc.scalar.activation(
            out=ex[:], in_=e7[:], func=mybir.ActivationFunctionType.Exp,
            bias=mn[:], scale=-1.0,
        )                                              # exp(rowmin - E)
        z = small.tile([NN, 1], F32)
        nc.vector.tensor_reduce(
            out=z[:], in_=ex[:], axis=mybir.AxisListType.X,
            op=mybir.AluOpType.add,
        )
        r = small.tile([NN, 1], F32)
        nc.vector.reciprocal(r[:], z[:])
        rg = small.tile([NN, 1], F32)
        nc.vector.tensor_mul(rg[:], r[:], gsb[:])      # gamma / Z_n
        a32 = small.tile([32, 32], F32)
        nc.vector.memset(a32[:], 0.0)
        nc.vector.tensor_scalar_mul(a32[0:NN, 0:NN], ex[:], rg[:])  # gamma*att
        at32 = small.tile([32, 32], F32)
        nc.vector.transpose(at32[:], a32[:])           # (gamma*att)^T
        # at16[k, 7q'+n] = at32[k, n]: broadcast along q', cast to fp16
        at16 = small.tile([NN, P], F16)
        nc.vector.tensor_copy(
            out=at16[:],
            in_=bass.AP(
                tensor=at32.tensor, offset=at32.offset,
                ap=[[at32.ap[0][0], NN], [0, Q], [1, NN]],
            ),
        )
        # W_ps[7q+m, 7q'+n] = at32[m, n] for every (q, q') block.
        # Reuses the gram bank again (e7 fully consumed by the exp above).
        w_ps = E[:]
        nc.tensor.matmul(w_ps, lhsT=l7[:], rhs=at16[:],
                         start=True, stop=True)
        # block-diag mask zeroes q != q' while copying PSUM -> SBUF fp16
        nc.vector.tensor_mul(wt[0:P, :], w_ps, mask[:])

        # ---------------- pass B: out = W.T @ Xh; y = out * x1 --------------
        for w in range(ws):
            x1t = x1s.tile([P, FS], F32)
            x1e = nc.scalar if w % 2 == 0 else nc.sync
            x1e.dma_start(out=x1t[:], in_=x1v[:, :, w, :])
            yt = ys.tile([P, FS], F32)
            for j in range(mpt):
                sl = slice(j * FM, (j + 1) * FM)
                op = ops.tile([P, FM], F32, tag="op")
                nc.tensor.matmul(op[:], lhsT=wt[:], rhs=xh[w][:, sl],
                                 start=True, stop=True)
                nc.vector.tensor_mul(yt[:, sl], op[:], x1t[:, sl])
            ye = (nc.gpsimd, nc.sync, nc.scalar)[w % 3]
            ye.dma_start(out=yv[:, :, w, :], in_=yt[:])

    nc.compile()
    return nc


_NC_CACHE = {}


def _get_nc(d_total=D_FULL):
    if d_total not in _NC_CACHE:
        _NC_CACHE[d_total] = build_nc(d_total)
    return _NC_CACHE[d_total]


def kernel(x1: np.ndarray, x2: np.ndarray, gamma: np.ndarray) -> np.ndarray:
    b, n, c, h, w = x1.shape
    assert (b, n) == (B, NN)
    d = c * h * w
    x1r = np.ascontiguousarray(x1.reshape(b, n, d)).astype(np.float32, copy=False)
    x2r = np.ascontiguousarray(x2.reshape(b, n, d)).astype(np.float32, copy=False)
    g = np.asarray(gamma, dtype=np.float32).reshape(1)

    nc = _get_nc(d)
    in_maps = [
        {"x1": x1r[i], "x2": x2r[i], "gamma": g} for i in range(N_CORES)
    ]
    res = run_bass_kernel_spmd(nc, in_maps, list(range(N_CORES)))
    out = np.stack([res.results[i]["y"] for i in range(N_CORES)], axis=0)
    return out.reshape(b, n, c, h, w).astype(np.float32, copy=False)
